# revision 20
# baseline (speedup 1.0000x reference)
"""Combined CE + Dice + Focal-Tversky segmentation loss on 8 Trainium2 cores.

Layout: pure data parallel, 2 images per core. Per image, class planes are
packed in "class pair" tiles [128, 4096] bf16: pair j holds class 2j on
partitions 0-63 and class 2j+1 on partitions 64-127; partition p%64 holds
pixels [(p%64)*4096, (p%64+1)*4096).

Engine split per image:
  ACT:  E = exp(lg) per pair, then Ln(S) per 512-sub (with fused per-sub lse
        accumulator columns), then one full-width R = exp(-lnS).  Ops are
        batched by function so the exp/ln ACT table sets do not thrash.
  PE:   S = cross-class sums (pair-sum+broadcast matmul into PSUM, per sub)
        and the per-class row-sums of q via one-hot ws matmuls.
  DVE:  per pair: lm = (tg==c)*lg as a fused scalar_tensor_tensor with a free
        accum_out row-sum (scheduled early - only needs the DMAed inputs),
        q = E*R as a 2x-mode tensor_tensor (in-place over E), and
        qm = (tg==c)*q as another fused scalar_tensor_tensor.
t_sum comes from a host-side bincount of the integer targets; the final
scalar combine runs on the host in float64 from the small [128, 16*bpc]
f32 stats tile DMAed out per core.
"""

import sys

sys.path.insert(0, "/opt/trn_rl_repo")

import numpy as np

import concourse.bacc as bacc
import concourse.mybir as mybir
import concourse.tile as tile
from concourse.bass_utils import run_bass_kernel_spmd

B, C, H, W = 16, 6, 512, 512
NCORES = 8
BPC = B // NCORES  # images per core
HWPX = H * W  # 262144 pixels per image
PHALF = 64
FD = HWPX // PHALF  # 4096 free-dim columns per image
NPAIR = C // 2  # 3 class-pair tiles

CE_W, DICE_W, FT_W = 0.4, 0.4, 0.2
FT_ALPHA, FT_BETA, FT_GAMMA = 0.7, 0.3, 1.33

BF16 = mybir.dt.bfloat16
F32 = mybir.dt.float32
AF = mybir.ActivationFunctionType
ALU = mybir.AluOpType
NPBF16 = mybir.dt.np(BF16)

SUB = 512  # PSUM-bank sub-chunk for matmuls
NSUB = FD // SUB
# stats cols per image: 0=q-fold (classes on partitions 0-5),
# 1+3*j+ci = qm accum per (pair, chunk), 10..13 = lse per 1024-col Ln
NACC = 16


def _build(fd=FD, sub=SUB, bpc=BPC):
    nsub = fd // sub
    nc = bacc.Bacc("TRN2", target_bir_lowering=False, debug=False,
                   enable_asserts=False, num_devices=NCORES)

    lg_d = nc.dram_tensor("lg", [bpc, NPAIR, 128, fd], BF16, kind="ExternalInput")
    tg_d = nc.dram_tensor("tg", [bpc, 128, fd], BF16, kind="ExternalInput")
    wd_d = nc.dram_tensor("wd", [128, 128], BF16, kind="ExternalInput")
    ws_d = nc.dram_tensor("ws", [128, NPAIR, 8], BF16, kind="ExternalInput")
    cv_d = nc.dram_tensor("cv", [128, NPAIR], F32, kind="ExternalInput")
    out_d = nc.dram_tensor("out", [128, bpc, NACC], F32, kind="ExternalOutput")

    with tile.TileContext(nc) as tc:
        with (
            tc.tile_pool(name="inp", bufs=1) as inp,
            tc.tile_pool(name="wk", bufs=2) as wk,
            tc.tile_pool(name="acc", bufs=1) as accp,
            tc.tile_pool(name="ps", bufs=3, space="PSUM") as ps,
            tc.tile_pool(name="pstat", bufs=1, space="PSUM") as pstat,
        ):
            wd_t = inp.tile([128, 128], BF16, tag="wd")
            nc.sync.dma_start(wd_t[:], wd_d.ap())
            ws_t = inp.tile([128, NPAIR, 8], BF16, tag="ws")
            nc.sync.dma_start(ws_t[:], ws_d.ap())
            cv_t = inp.tile([128, NPAIR], F32, tag="cv")
            nc.sync.dma_start(cv_t[:], cv_d.ap())

            # descending/ascending chunk widths: a small first chunk shortens
            # the pipeline ramp, a small last chunk shortens the DVE tail
            chunks = {0: (1024, 1024, 2048), 1: (2048, 1024, 1024)}
            lg_t = inp.tile([128, bpc, NPAIR, fd], BF16, tag="lg")
            tg_t = inp.tile([128, bpc, fd], BF16, tag="tg")
            for b in range(bpc):
                for j in range(NPAIR):
                    off = 0
                    for cw in chunks[b]:
                        csl = slice(off, off + cw)
                        nc.sync.dma_start(lg_t[:, b, j, csl],
                                          lg_d.ap()[b, j][:, csl])
                        off += cw
                nc.sync.dma_start(tg_t[:, b, :], tg_d.ap()[b])

            out_sb = accp.tile([128, bpc, NACC], F32, tag="out")
            nc.vector.memset(out_sb[:], 0.0)

            for b in range(bpc):
                junk = wk.tile([128, fd], BF16, tag="junk")
                E = wk.tile([128, NPAIR, fd], BF16, tag="E")
                R2 = wk.tile([128, fd], BF16, tag="R2")
                st_q = pstat.tile([8, sub], F32, tag="st_q")
                off = 0
                lncol = 10
                for ci, cw in enumerate(chunks[b]):
                    csl = slice(off, off + cw)
                    # E = exp(logits), one 2D op per (pair, chunk)
                    for j in range(NPAIR):
                        nc.scalar.activation(
                            E[:, j, csl], lg_t[:, b, j, csl], AF.Exp)
                    # S into 2-bank PSUM pieces; one Ln per 1024 cols
                    lse = wk.tile([128, cw], F32, tag=f"lse{ci}")
                    for k in range(cw // 1024):
                        s2 = ps.tile([128, 1024], F32, tag="s2")
                        for s in range(2):
                            esl = slice(off + k * 1024 + s * sub,
                                        off + k * 1024 + (s + 1) * sub)
                            for j in range(NPAIR):
                                nc.tensor.matmul(
                                    s2[:, s * sub:(s + 1) * sub], wd_t[:],
                                    E[:, j, esl],
                                    start=(j == 0), stop=(j == NPAIR - 1),
                                )
                        nc.scalar.activation(
                            lse[:, k * 1024:(k + 1) * 1024], s2[:], AF.Ln,
                            accum_out=out_sb[:, b, lncol:lncol + 1])
                        lncol += 1
                    # R = 1/S for this chunk
                    nc.scalar.activation(
                        R2[:, csl], lse[:], AF.Exp, scale=-1.0)
                    # q = E*R in-place; q row-sums on PE; qm fused on DVE
                    for j in range(NPAIR):
                        nc.vector.tensor_tensor(
                            E[:, j, csl], E[:, j, csl], R2[:, csl], ALU.mult)
                        for s in range(cw // sub):
                            esl = slice(off + s * sub, off + (s + 1) * sub)
                            nc.tensor.matmul(
                                st_q[:], ws_t[:, j, :], E[:, j, esl],
                                start=(ci == 0 and j == 0 and s == 0),
                                stop=(ci == len(chunks[b]) - 1
                                      and j == NPAIR - 1
                                      and s == cw // sub - 1))
                        qmc = 1 + 3 * j + ci
                        nc.vector.scalar_tensor_tensor(
                            out=junk[:, csl], in0=tg_t[:, b, csl],
                            scalar=cv_t[:, j:j + 1], in1=E[:, j, csl],
                            op0=ALU.is_equal, op1=ALU.mult,
                            accum_out=out_sb[:, b, qmc:qmc + 1])
                    off += cw
                nc.vector.tensor_reduce(
                    out_sb[0:8, b, 0:1], st_q[:],
                    axis=mybir.AxisListType.X, op=ALU.add)
            nc.sync.dma_start(out_d.ap(), out_sb[:])
    nc.compile()
    return nc


def _weights():
    k = np.arange(128)
    wd = (k[:, None] % 64 == k[None, :] % 64).astype(NPBF16)
    ws = np.zeros((128, NPAIR, 8), dtype=NPBF16)
    for j in range(NPAIR):
        ws[:64, j, 2 * j] = 1
        ws[64:, j, 2 * j + 1] = 1
    cv = np.zeros((128, NPAIR), dtype=np.float32)
    for j in range(NPAIR):
        cv[:64, j] = 2 * j
        cv[64:, j] = 2 * j + 1
    return wd, ws, cv


def _prep_core(logits_np, targets_np, cores, bpc, fd):
    """Build per-core input maps. logits (B,C,H,W) f32, targets (B,H,W) int."""
    wd, ws, cv = _weights()
    lg = np.ascontiguousarray(logits_np.reshape(B, NPAIR, 128, fd)).astype(NPBF16)
    tghalf = targets_np.reshape(B, PHALF, fd).astype(NPBF16)
    tg = np.concatenate([tghalf, tghalf], axis=1)  # duplicate to both halves
    maps = []
    for c in range(cores):
        maps.append({
            "lg": np.ascontiguousarray(lg[c * bpc:(c + 1) * bpc]),
            "tg": np.ascontiguousarray(tg[c * bpc:(c + 1) * bpc]),
            "wd": wd, "ws": ws, "cv": cv,
        })
    return maps


def _finish(outs, targets_np, bpc, logits_bf16=None):
    """Host combine: outs = list of [128, bpc, NACC] f32 per core.

    logits_bf16: [B, C, HWPX] logits view; the CE numerator (sum of
    target-class logits) is a pure input gather, done here.
    """
    p_sum = np.zeros((B, C)); tp = np.zeros((B, C))
    lse = np.zeros(B)
    for core, o in enumerate(outs):
        o = o.astype(np.float64)
        for b in range(bpc):
            img = core * bpc + b
            p_sum[img] = o[0:6, b, 0]
            for j in range(NPAIR):
                cols = slice(1 + 3 * j, 4 + 3 * j)
                tp[img, 2 * j] = o[:64, b, cols].sum()
                tp[img, 2 * j + 1] = o[64:, b, cols].sum()
            lse[img] = o[:, b, 10:14].sum() / 2.0
    t_sum = np.stack([np.bincount(targets_np[i].ravel().astype(np.int64),
                                  minlength=C).astype(np.float64)
                      for i in range(B)])
    tflat = targets_np.reshape(B, 1, HWPX).astype(np.int64)
    xt = np.take_along_axis(logits_bf16, tflat, axis=1).sum(dtype=np.float64)
    npx = B * HWPX
    ce = (lse.sum() - xt) / npx
    dice = (2.0 * tp + 1e-8) / (p_sum + t_sum + 1e-8)
    dice_loss = np.mean(1.0 - dice)
    fp = p_sum - tp
    fn = t_sum - tp
    tversky = (tp + 1e-6) / (tp + FT_ALPHA * fn + FT_BETA * fp + 1e-6)
    ft_loss = np.mean((1.0 - tversky) ** FT_GAMMA)
    return np.float32(CE_W * ce + DICE_W * dice_loss + FT_W * ft_loss)


_CACHED = {}


def kernel(logits, targets):
    logits = np.asarray(logits, dtype=np.float32)
    targets = np.asarray(targets)
    if "nc" not in _CACHED:
        _CACHED["nc"] = _build()
    maps = _prep_core(logits, targets, NCORES, BPC, FD)
    res = run_bass_kernel_spmd(_CACHED["nc"], maps, list(range(NCORES)))
    outs = [res.results[i]["out"] for i in range(NCORES)]
    return _finish(outs, targets, BPC, logits.reshape(B, C, HWPX))


if __name__ == "__main__":
    rng = np.random.default_rng(0)
    logits = rng.standard_normal((B, C, H, W), dtype=np.float32)
    targets = rng.integers(0, C, size=(B, H, W)).astype(np.int64)
    got = kernel(logits, targets)

    # float64 numpy reference
    lg = logits.astype(np.float64)
    m = lg.max(axis=1, keepdims=True)
    e = np.exp(lg - m)
    s = e.sum(axis=1, keepdims=True)
    logp = lg - m - np.log(s)
    probs = e / s
    lp_t = np.take_along_axis(logp, targets[:, None], axis=1)[:, 0]
    ce = -lp_t.mean()
    oh = (targets[:, None] == np.arange(C)[None, :, None, None])
    tp = (probs * oh).sum(axis=(2, 3))
    p_sum = probs.sum(axis=(2, 3))
    t_sum = oh.sum(axis=(2, 3))
    dice = (2 * tp + 1e-8) / (p_sum + t_sum + 1e-8)
    dice_loss = np.mean(1 - dice)
    tv = (tp + 1e-6) / (tp + FT_ALPHA * (t_sum - tp) + FT_BETA * (p_sum - tp) + 1e-6)
    ft = np.mean((1 - tv) ** FT_GAMMA)
    want = CE_W * ce + DICE_W * dice_loss + FT_W * ft
    print("got", got, "want", want, "rel", abs(got - want) / abs(want))
